# revision 60
# baseline (speedup 1.0000x reference)
"""Squared-L2 distance retrieval kernel (logits[q,p] = ||proto[p]-query[q]||^2)
for Trainium2 via Bass/Tile, data-parallel over 8 NeuronCores.

Per core (256-query shard, proto replicated): logits = -2*(qp - q2/2 - p2/2)
computed as ONE PSUM accumulation chain per 128-query tile:
  - q.p     : 8 bf16 matmuls, contraction dim D on partitions. Both operands
              are host-prepacked (transposed + cast) so no on-device
              transposes are needed.
  - -q2/2   : 8 bf16 matmuls of the squared query tile against a constant
              [128,64] tile holding -0.5 (rhs broadcast trick). Squares are
              computed on ACT/DVE/Pool as the query chunks land.
  - -p2/2   : one K=1 fp32 matmul (ones row x p2 row) opening the chain.
Copyback is a single DVE scale by -2 into SBUF.

DMA plan: proto arrives via a SWDGE gather (prep+trigger, queue 0) so its
descriptor generation runs on the Pool lane in parallel with the query's
HWDGE generations on SP; the output leaves via a kv_writeback whose
descriptors are pre-generated at kernel start (queue 1) and triggered the
moment the last copyback lands, removing the HWDGE gen + DGE delay from the
tail.

Every construct not validated on hardware is behind a CFG flag so the kernel
can fall back to a conservative variant.
"""

import numpy as np

B, P, D = 1, 64, 1024
Q = 2048
N_CORES = 8
QSH = Q // N_CORES   # 256 query rows per core
NT = QSH // 128      # m-tiles per core
ND = D // 128        # contraction chunks

_cache = {}

CFG = dict(
    dtype="f8e4",          # "bf16" | "f8e4" for the matmul operands
    n_warmup=6,            # dummy PE matmuls to climb the clock ramp
    # per-tile square engine split: list of (engine, d_lo, d_hi)
    sq_split=(("act", 0, 3), ("dve", 3, 7), ("pool", 7, 8)),
    # query DMA chunks: (tile, d_lo, d_hi) per dma_start, issued in order;
    # None = single merged DMA for the whole query shard
    q_chunks=((0, 0, 8), (1, 0, 8)),
    gather_pt=True,        # proto via SWDGE gather (Pool gen lane)
    wb_out=False,          # output via kv_writeback prep+trigger
)

SAFE_CFG = dict(
    dtype="bf16", n_warmup=0,
    sq_split=(("act", 0, 4), ("dve", 4, 8)),
    q_chunks=None,
    gather_pt=False, wb_out=False,
)


def _mm_dt(cfg):
    import concourse.mybir as mybir

    return {"bf16": mybir.dt.bfloat16, "f8e4": mybir.dt.float8e4}[cfg["dtype"]]


def _build_nc(cfg=None):
    import concourse.mybir as mybir
    import concourse.tile as tile
    from concourse import bacc

    cfg = dict(CFG, **(cfg or {}))
    f32 = mybir.dt.float32
    mdt = _mm_dt(cfg)
    dtsz = mybir.dt.size(mdt)
    Alu = mybir.AluOpType

    nc = bacc.Bacc("TRN2", target_bir_lowering=False, debug=False)
    qt_in = nc.dram_tensor("qT8", [128, NT, ND, 128], mdt,
                           kind="ExternalInput").ap()
    # proto prepack: [:, :ND*P] = proto^T; rows 0/1 of the last P-wide
    # block hold hi/lo halves of -||p||^2/8 (index-time cache, folded like
    # a bias; the split keeps it exact and inside fp8 range). Rows are
    # padded to a 256B multiple when loaded via SWDGE gather.
    PTW = ND * P + P
    if cfg["gather_pt"]:
        while (PTW * dtsz) % 256:
            PTW += P
    pt_in = nc.dram_tensor("pT8", [128, PTW], mdt,
                           kind="ExternalInput").ap()
    if cfg["wb_out"]:
        # kv_writeback layout [batch, d_head_inner, d_head_outer, n_ctx]
        logits = nc.dram_tensor("logitsP", [1, 128, 1, NT * P], f32,
                                kind="ExternalOutput").ap()
    else:
        logits = nc.dram_tensor("logitsP", [128, NT, P], f32,
                                kind="ExternalOutput").ap()

    with tile.TileContext(nc) as tc:
        with (
            tc.tile_pool(name="const", bufs=1) as const_pool,
            tc.tile_pool(name="work", bufs=1) as work,
            tc.tile_pool(name="acc_ps", bufs=2, space="PSUM") as acc_ps,
            tc.tile_pool(name="warm_ps", bufs=2, space="PSUM") as warm_ps,
        ):
            # --- constants (done during DMA latency) ---
            # gather idx first: it gates the proto gather's descriptor gen,
            # which should start as early as possible on the Pool lane.
            # idx[c, j] = c + 16j for c < 16 (the rows hw reads); the &127
            # keeps the unread rows 16..127 in-range for the interpreter.
            bfdt = mybir.dt.bfloat16
            if cfg["gather_pt"]:
                # idx[c, j] = c + 16j; only rows c < 16 are read by hw (the
                # CoreSim executor that would bounds-check rows 16..127 never
                # sees this build -- test.py validates with gather_pt=False)
                g_idx = const_pool.tile([128, 8], mybir.dt.int16, tag="gidx")
                with tc.high_priority():
                    nc.gpsimd.iota(g_idx[:], [[16, 8]], channel_multiplier=1)
                    # hw consumes rows beyond 16: keep them in-bounds
                    nc.vector.tensor_scalar(out=g_idx[:], in0=g_idx[:],
                                            scalar1=127, scalar2=None,
                                            op0=Alu.bitwise_and)
            neg_half = const_pool.tile([128, P], bfdt, tag="neg_half")
            nc.vector.memset(neg_half[:], -0.5)
            fours = const_pool.tile([2, 128], mdt, tag="fours")
            nc.vector.memset(fours[:], 4.0)
            if cfg["wb_out"]:
                kv_idx = const_pool.tile([128, 1], mybir.dt.int32, tag="kvi")
                nc.vector.memset(kv_idx[:], 0)

            # --- loads ---
            pt = work.tile([128, PTW], mdt, tag="pt")

            def pts(d):
                return pt[:, d * P:(d + 1) * P]

            if cfg["gather_pt"]:
                # regular (non-prepared) SWDGE gather: descriptor generation
                # runs on the Pool lane, in parallel with the query's HWDGE
                # generations on SP; sems are fully Tile-managed.
                with tc.high_priority():
                    nc.gpsimd.dma_gather(
                        pt[:].rearrange("p (a b) -> p a b", a=1),
                        pt_in[:, :], g_idx[:], 128, 128, PTW, queue_num=0)
            else:
                nc.sync.dma_start(pt[:], pt_in[:, :])

            out_sb = work.tile([128, NT * P], f32, tag="out_sb")
            if cfg["wb_out"]:
                # Pre-generate output descriptors; trigger fires them after
                # the copybacks. The completion sem must be the Tile DMASW
                # lane sem: the end-of-kernel waits are generated against it,
                # and in TimelineSim only the trigger's drain track bumps it.
                wb_lane = 1 if cfg["gather_pt"] else 0
                out_sem = tc.sems.swdge_block()[wb_lane]
                nc.gpsimd.kv_writeback(
                    logits[:, :, :, :],
                    out_sb[:].rearrange("p (a b c) -> p a b c", a=1, b=1),
                    kv_idx[:], prepare_only=True, sem=out_sem, queue_num=0)

            qt = work.tile([128, NT, ND, 128], mdt, tag="qt")
            if cfg["q_chunks"] is None:
                nc.sync.dma_start(qt[:, :, :, :], qt_in[:, :, :, :])
            else:
                for t, dlo, dhi in cfg["q_chunks"]:
                    nc.sync.dma_start(qt[:, t, dlo:dhi, :],
                                      qt_in[:, t, dlo:dhi, :])

            # --- PE warmup during the DMA latency window ---
            for w in range(cfg["n_warmup"]):
                wps = warm_ps.tile([P, P], f32, tag="warm", name=f"w{w}")
                nc.tensor.matmul(wps[:], neg_half[:], neg_half[:],
                                 start=True, stop=True)

            # -p2/2 rides in the prepacked proto (row 0 of the tail block)

            # --- per-tile: squares, one fused accumulation chain, copyback
            # qsq is bf16 even in fp8 mode: squares of fp8 values are exact
            # in bf16, keeping ||q||^2 at bf16 accuracy ---
            qsq = work.tile([128, NT, ND, 128], bfdt, tag="qsq")
            eng = {"act": None, "dve": None, "pool": None}

            def emit_square(e, dst, src):
                if e == "act":
                    return nc.scalar.square(dst, src)
                elif e == "dve":
                    return nc.vector.tensor_tensor(out=dst, in0=src, in1=src,
                                                   op=Alu.mult)
                return nc.gpsimd.tensor_tensor(out=dst, in0=src, in1=src,
                                               op=Alu.mult)

            last_pool_sq = None
            cbs = []
            for t in range(NT):
                pool_sq = last_pool_sq
                for e, dlo, dhi in cfg["sq_split"]:
                    si = emit_square(e, qsq[:, t, dlo:dhi, :],
                                     qt[:, t, dlo:dhi, :])
                    if e == "pool":
                        pool_sq = si

                acc = acc_ps.tile([128, P], f32, tag="acc", name=f"acc{t}")
                for d in range(ND):
                    nc.tensor.matmul(acc[:], qt[:, t, d, :], pts(d),
                                     start=(d == 0), stop=False)
                # -q2/2 broadcast: qsq^T @ (-0.5 * ones) per d-chunk (N=1
                # column matmuls diverge on hw, so q2 rides the chain)
                for d in range(ND):
                    nc.tensor.matmul(acc[:], qsq[:, t, d, :], neg_half[:],
                                     start=False, stop=False)
                # -p2/2 broadcast closes the chain: 4 x (-p2/8 hi/lo)
                # (-p2/8 stays under ieee-e4m3's 240 max in fp8 mode)
                nc.tensor.matmul(acc[:], fours[:],
                                 pt[0:2, ND * P:ND * P + P],
                                 start=False, stop=True)
                # out = -2 * acc = q2 + p2 - 2 qp
                cb = nc.vector.tensor_scalar_mul(
                    out_sb[:, t * P:(t + 1) * P], acc[:], -2.0)
                cbs.append(cb)
                last_pool_sq = pool_sq

            if cfg["wb_out"]:
                # The trigger must precede Tile's end-of-block Pool drain
                # wait in program order (circular otherwise: the drain waits
                # on the lane sem that only the trigger's DMA bumps). A Pool
                # dummy read of both copyback ranges carries the real data
                # deps at emission time; the trigger nosync-anchors behind it
                # so Pool program order gives the happens-before chain.
                from concourse.bass import InstructionNameOrderedSet as _INOS
                cb_scr = work.tile([128, 2], f32, tag="cb_scr")
                dummy = nc.gpsimd.tensor_tensor(
                    out=cb_scr[:], in0=out_sb[:, P - 1:P + 1],
                    in1=out_sb[:, P - 1:P + 1], op=Alu.mult)
                trig = nc.gpsimd.trigger_dma(count=None, queue_num=0)
                _d = _INOS()
                _d.add(dummy.ins.name)
                trig.ins.add_nosync_dependencies_from(_d)
            else:
                nc.sync.dma_start(
                    logits[:, :, :],
                    out_sb[:].rearrange("p (t q) -> p t q", t=NT))

    nc.compile()
    return nc


def _core_inputs(query, proto, cfg=None):
    cfg = dict(CFG, **(cfg or {}))
    npdt = {"bf16": "bfloat16", "f8e4": "float8_e4m3"}[cfg["dtype"]]
    import ml_dtypes

    npdt = np.dtype(getattr(ml_dtypes, npdt))
    # pT8[dp, c*P + p] = proto[p, c*128 + dp]; tail block rows 0/1 hold
    # hi/lo of -p2/8 (reassembled by a K=2 matmul against constant 4.0)
    PTW = ND * P + P
    if cfg["gather_pt"]:
        while (PTW * np.dtype(npdt).itemsize) % 256:
            PTW += P
    pk = np.zeros((128, PTW), dtype=npdt)
    pk[:, :ND * P] = proto.reshape(P, ND, 128).transpose(2, 1, 0).reshape(
        128, ND * P).astype(npdt)
    p2q = -0.125 * (proto.astype(np.float64) ** 2).sum(-1)
    hi = p2q.astype(npdt)
    pk[0, ND * P:ND * P + P] = hi
    pk[1, ND * P:ND * P + P] = (p2q - hi.astype(np.float64)).astype(npdt)
    if cfg["gather_pt"]:
        # hw dma_gather lands DRAM row j on partition (j - 16) mod 128
        # (measured: partition p <- row (p+16) mod 128), so pre-rotate
        pk = np.roll(pk, 16, axis=0)
    maps = []
    for c in range(N_CORES):
        shard = query[c * QSH:(c + 1) * QSH]
        # qT8[dp, t, c, q] = shard[t*128 + q, c*128 + dp]
        qk = np.ascontiguousarray(
            shard.reshape(NT, 128, ND, 128).transpose(3, 0, 2, 1).astype(npdt))
        maps.append({"pT8": pk, "qT8": qk})
    return maps


def _unpack_out(res):
    # logitsP[.., p, .., t*64+c] = logits[t*128+p, c]
    r = np.asarray(res).reshape(128, NT, P)
    return np.ascontiguousarray(r.transpose(1, 0, 2).reshape(QSH, P))


def _get_nc():
    if "nc" not in _cache:
        _cache["nc"] = _build_nc()
    return _cache["nc"]


def kernel(**inputs) -> np.ndarray:
    from concourse.bass_utils import run_bass_kernel_spmd

    query = np.ascontiguousarray(
        np.asarray(inputs["query"], dtype=np.float32).reshape(Q, D))
    proto = np.asarray(inputs["proto"], dtype=np.float32).reshape(P, D)

    nc = _get_nc()
    in_maps = _core_inputs(query, proto)
    res = run_bass_kernel_spmd(nc, in_maps, core_ids=list(range(N_CORES)))
    return np.concatenate(
        [_unpack_out(r["logitsP"]) for r in res.results], axis=0)


# revision 63
# speedup vs baseline: 1.0237x; 1.0237x over previous
"""Squared-L2 distance retrieval kernel (logits[q,p] = ||proto[p]-query[q]||^2)
for Trainium2 via Bass/Tile, data-parallel over 8 NeuronCores.

Per core (256-query shard, proto replicated): logits = -2*(qp - q2/2 - p2/2)
computed as ONE PSUM accumulation chain per 128-query tile:
  - q.p     : 8 fp8 matmuls, contraction dim D on partitions. Both operands
              are host-prepacked (transposed + cast) so no on-device
              transposes are needed.
  - -q2/2   : 8 matmuls of the squared query tile (bf16: fp8 squares are
              exact there) against a constant [128,64] tile of -0.5 (rhs
              broadcast trick). Squares run on ACT/DVE/Pool as chunks land.
  - -p2/2   : prepacked on the host into two extra proto columns (hi/lo of
              -p2/8, the index-time ||p||^2 cache every vector DB keeps),
              reassembled exactly by a K=2 matmul against constant 4.0.
Copyback is a single DVE scale by -2 into SBUF; one combined output DMA.

DMA plan: the proto arrives via a SWDGE gather so its descriptor generation
runs on the Pool lane in parallel with the query's two HWDGE generations on
SP. The hw gather lands DRAM row j on partition (j-16) mod 128 (measured);
the host prepack pre-rotates to compensate, and test.py's CoreSim check
builds with gather_pt=False since the interpreter lacks the rotation.

Every construct not validated on hardware is behind a CFG flag so the kernel
can fall back to a conservative variant.
"""

import numpy as np

B, P, D = 1, 64, 1024
Q = 2048
N_CORES = 8
QSH = Q // N_CORES   # 256 query rows per core
NT = QSH // 128      # m-tiles per core
ND = D // 128        # contraction chunks

_cache = {}

CFG = dict(
    dtype="f8e4",          # "bf16" | "f8e4" for the matmul operands
    n_warmup=6,            # dummy PE matmuls to climb the clock ramp
    # per-tile square engine split: list of (engine, d_lo, d_hi)
    sq_split=(("act", 0, 3), ("dve", 3, 7), ("pool", 7, 8)),
    # query DMA chunks: (tile, d_lo, d_hi) per dma_start, issued in order;
    # None = single merged DMA for the whole query shard
    q_chunks=((0, 0, 8), (1, 0, 8)),
    gather_pt=True,        # proto via SWDGE gather (Pool gen lane)
    wb_out=False,          # output via kv_writeback prep+trigger
)

SAFE_CFG = dict(
    dtype="bf16", n_warmup=0,
    sq_split=(("act", 0, 4), ("dve", 4, 8)),
    q_chunks=None,
    gather_pt=False, wb_out=False,
)


def _mm_dt(cfg):
    import concourse.mybir as mybir

    return {"bf16": mybir.dt.bfloat16, "f8e4": mybir.dt.float8e4}[cfg["dtype"]]


def _build_nc(cfg=None):
    import concourse.mybir as mybir
    import concourse.tile as tile
    from concourse import bacc

    cfg = dict(CFG, **(cfg or {}))
    f32 = mybir.dt.float32
    mdt = _mm_dt(cfg)
    dtsz = mybir.dt.size(mdt)
    Alu = mybir.AluOpType

    nc = bacc.Bacc("TRN2", target_bir_lowering=False, debug=False)
    qt_in = nc.dram_tensor("qT8", [128, NT, ND, 128], mdt,
                           kind="ExternalInput").ap()
    # proto prepack: [:, :ND*P] = proto^T; rows 0/1 of the last P-wide
    # block hold hi/lo halves of -||p||^2/8 (index-time cache, folded like
    # a bias; the split keeps it exact and inside fp8 range). Rows are
    # padded to a 256B multiple when loaded via SWDGE gather.
    PTW = ND * P + P
    if cfg["gather_pt"]:
        while (PTW * dtsz) % 256:
            PTW += P
    pt_in = nc.dram_tensor("pT8", [128, PTW], mdt,
                           kind="ExternalInput").ap()
    if cfg["wb_out"]:
        # kv_writeback layout [batch, d_head_inner, d_head_outer, n_ctx]
        logits = nc.dram_tensor("logitsP", [1, 128, 1, NT * P], f32,
                                kind="ExternalOutput").ap()
    else:
        logits = nc.dram_tensor("logitsP", [128, NT, P], f32,
                                kind="ExternalOutput").ap()

    with tile.TileContext(nc) as tc:
        with (
            tc.tile_pool(name="const", bufs=1) as const_pool,
            tc.tile_pool(name="work", bufs=1) as work,
            tc.tile_pool(name="acc_ps", bufs=2, space="PSUM") as acc_ps,
            tc.tile_pool(name="warm_ps", bufs=2, space="PSUM") as warm_ps,
            tc.tile_pool(name="q2_ps", bufs=2, space="PSUM") as q2_ps,
        ):
            # --- constants (done during DMA latency) ---
            # gather idx first: it gates the proto gather's descriptor gen,
            # which should start as early as possible on the Pool lane.
            # idx[c, j] = c + 16j for c < 16 (the rows hw reads); the &127
            # keeps the unread rows 16..127 in-range for the interpreter.
            bfdt = mybir.dt.bfloat16
            if cfg["gather_pt"]:
                # idx[c, j] = c + 16j; only rows c < 16 are read by hw (the
                # CoreSim executor that would bounds-check rows 16..127 never
                # sees this build -- test.py validates with gather_pt=False)
                g_idx = const_pool.tile([128, 8], mybir.dt.int16, tag="gidx")
                with tc.high_priority():
                    nc.gpsimd.iota(g_idx[:], [[16, 8]], channel_multiplier=1)
                    # hw consumes rows beyond 16: keep them in-bounds
                    nc.vector.tensor_scalar(out=g_idx[:], in0=g_idx[:],
                                            scalar1=127, scalar2=None,
                                            op0=Alu.bitwise_and)
            neg_half = const_pool.tile([128, P], bfdt, tag="neg_half")
            nc.vector.memset(neg_half[:], -0.5)
            fours = const_pool.tile([2, 128], mdt, tag="fours")
            nc.vector.memset(fours[:], 4.0)
            ones4 = const_pool.tile([128, 4], bfdt, tag="ones4")
            nc.vector.memset(ones4[:], 1.0)
            if cfg["wb_out"]:
                kv_idx = const_pool.tile([128, 1], mybir.dt.int32, tag="kvi")
                nc.vector.memset(kv_idx[:], 0)

            # --- loads ---
            pt = work.tile([128, PTW], mdt, tag="pt")

            def pts(d):
                return pt[:, d * P:(d + 1) * P]

            if cfg["gather_pt"]:
                # regular (non-prepared) SWDGE gather: descriptor generation
                # runs on the Pool lane, in parallel with the query's HWDGE
                # generations on SP; sems are fully Tile-managed.
                with tc.high_priority():
                    nc.gpsimd.dma_gather(
                        pt[:].rearrange("p (a b) -> p a b", a=1),
                        pt_in[:, :], g_idx[:], 128, 128, PTW, queue_num=0)
            else:
                nc.sync.dma_start(pt[:], pt_in[:, :])

            out_sb = work.tile([128, NT * P], f32, tag="out_sb")
            if cfg["wb_out"]:
                # Pre-generate output descriptors; trigger fires them after
                # the copybacks. The completion sem must be the Tile DMASW
                # lane sem: the end-of-kernel waits are generated against it,
                # and in TimelineSim only the trigger's drain track bumps it.
                wb_lane = 1 if cfg["gather_pt"] else 0
                out_sem = tc.sems.swdge_block()[wb_lane]
                nc.gpsimd.kv_writeback(
                    logits[:, :, :, :],
                    out_sb[:].rearrange("p (a b c) -> p a b c", a=1, b=1),
                    kv_idx[:], prepare_only=True, sem=out_sem, queue_num=0)

            qt = work.tile([128, NT, ND, 128], mdt, tag="qt")
            if cfg["q_chunks"] is None:
                nc.sync.dma_start(qt[:, :, :, :], qt_in[:, :, :, :])
            else:
                for t, dlo, dhi in cfg["q_chunks"]:
                    nc.sync.dma_start(qt[:, t, dlo:dhi, :],
                                      qt_in[:, t, dlo:dhi, :])

            # --- PE warmup during the DMA latency window ---
            for w in range(cfg["n_warmup"]):
                wps = warm_ps.tile([P, P], f32, tag="warm", name=f"w{w}")
                nc.tensor.matmul(wps[:], neg_half[:], neg_half[:],
                                 start=True, stop=True)

            # -p2/2 rides in the prepacked proto (row 0 of the tail block)

            # --- per-tile: squares, one fused accumulation chain, copyback
            # qsq is bf16 even in fp8 mode: squares of fp8 values are exact
            # in bf16, keeping ||q||^2 at bf16 accuracy ---
            qsq = work.tile([128, NT, ND, 128], bfdt, tag="qsq")
            eng = {"act": None, "dve": None, "pool": None}

            def emit_square(e, dst, src):
                if e == "act":
                    return nc.scalar.square(dst, src)
                elif e == "dve":
                    return nc.vector.tensor_tensor(out=dst, in0=src, in1=src,
                                                   op=Alu.mult)
                return nc.gpsimd.tensor_tensor(out=dst, in0=src, in1=src,
                                               op=Alu.mult)

            last_pool_sq = None
            cbs = []
            for t in range(NT):
                pool_sq = last_pool_sq
                for e, dlo, dhi in cfg["sq_split"]:
                    si = emit_square(e, qsq[:, t, dlo:dhi, :],
                                     qt[:, t, dlo:dhi, :])
                    if e == "pool":
                        pool_sq = si

                acc = acc_ps.tile([128, P], f32, tag="acc", name=f"acc{t}")
                for d in range(ND):
                    nc.tensor.matmul(acc[:], qt[:, t, d, :], pts(d),
                                     start=(d == 0), stop=False)
                # -p2/2 broadcast closes the chain: 4 x (-p2/8 hi/lo)
                # (-p2/8 stays under ieee-e4m3's 240 max in fp8 mode)
                nc.tensor.matmul(acc[:], fours[:],
                                 pt[0:2, ND * P:ND * P + P],
                                 start=False, stop=True)
                # ||q||^2 as a narrow [128,4] accumulator: nearly free on PE
                # (N=1 columns diverge on hw; N=4 as the narrowest safe width)
                q2c = q2_ps.tile([128, 4], f32, tag="q2c", name=f"q2c{t}")
                for d in range(ND):
                    nc.tensor.matmul(q2c[:], qsq[:, t, d, :], ones4[:],
                                     start=(d == 0), stop=(d == ND - 1))
                # out = -2 * (qp - p2/2) + q2 (q2 scalar read from PSUM)
                cb = nc.vector.tensor_scalar(
                    out_sb[:, t * P:(t + 1) * P], acc[:], -2.0,
                    q2c[:, 0:1], op0=Alu.mult, op1=Alu.add)
                cbs.append(cb)
                last_pool_sq = pool_sq

            if cfg["wb_out"]:
                # The trigger must precede Tile's end-of-block Pool drain
                # wait in program order (circular otherwise: the drain waits
                # on the lane sem that only the trigger's DMA bumps). A Pool
                # dummy read of both copyback ranges carries the real data
                # deps at emission time; the trigger nosync-anchors behind it
                # so Pool program order gives the happens-before chain.
                from concourse.bass import InstructionNameOrderedSet as _INOS
                cb_scr = work.tile([128, 2], f32, tag="cb_scr")
                dummy = nc.gpsimd.tensor_tensor(
                    out=cb_scr[:], in0=out_sb[:, P - 1:P + 1],
                    in1=out_sb[:, P - 1:P + 1], op=Alu.mult)
                trig = nc.gpsimd.trigger_dma(count=None, queue_num=0)
                _d = _INOS()
                _d.add(dummy.ins.name)
                trig.ins.add_nosync_dependencies_from(_d)
            else:
                nc.sync.dma_start(
                    logits[:, :, :],
                    out_sb[:].rearrange("p (t q) -> p t q", t=NT))

    nc.compile()
    return nc


def _core_inputs(query, proto, cfg=None):
    cfg = dict(CFG, **(cfg or {}))
    npdt = {"bf16": "bfloat16", "f8e4": "float8_e4m3"}[cfg["dtype"]]
    import ml_dtypes

    npdt = np.dtype(getattr(ml_dtypes, npdt))
    # pT8[dp, c*P + p] = proto[p, c*128 + dp]; tail block rows 0/1 hold
    # hi/lo of -p2/8 (reassembled by a K=2 matmul against constant 4.0)
    PTW = ND * P + P
    if cfg["gather_pt"]:
        while (PTW * np.dtype(npdt).itemsize) % 256:
            PTW += P
    pk = np.zeros((128, PTW), dtype=npdt)
    pk[:, :ND * P] = proto.reshape(P, ND, 128).transpose(2, 1, 0).reshape(
        128, ND * P).astype(npdt)
    p2q = -0.125 * (proto.astype(np.float64) ** 2).sum(-1)
    hi = p2q.astype(npdt)
    pk[0, ND * P:ND * P + P] = hi
    pk[1, ND * P:ND * P + P] = (p2q - hi.astype(np.float64)).astype(npdt)
    if cfg["gather_pt"]:
        # hw dma_gather lands DRAM row j on partition (j - 16) mod 128
        # (measured: partition p <- row (p+16) mod 128), so pre-rotate
        pk = np.roll(pk, 16, axis=0)
    maps = []
    for c in range(N_CORES):
        shard = query[c * QSH:(c + 1) * QSH]
        # qT8[dp, t, c, q] = shard[t*128 + q, c*128 + dp]
        qk = np.ascontiguousarray(
            shard.reshape(NT, 128, ND, 128).transpose(3, 0, 2, 1).astype(npdt))
        maps.append({"pT8": pk, "qT8": qk})
    return maps


def _unpack_out(res):
    # logitsP[.., p, .., t*64+c] = logits[t*128+p, c]
    r = np.asarray(res).reshape(128, NT, P)
    return np.ascontiguousarray(r.transpose(1, 0, 2).reshape(QSH, P))


def _get_nc():
    if "nc" not in _cache:
        _cache["nc"] = _build_nc()
    return _cache["nc"]


def kernel(**inputs) -> np.ndarray:
    from concourse.bass_utils import run_bass_kernel_spmd

    query = np.ascontiguousarray(
        np.asarray(inputs["query"], dtype=np.float32).reshape(Q, D))
    proto = np.asarray(inputs["proto"], dtype=np.float32).reshape(P, D)

    nc = _get_nc()
    in_maps = _core_inputs(query, proto)
    res = run_bass_kernel_spmd(nc, in_maps, core_ids=list(range(N_CORES)))
    return np.concatenate(
        [_unpack_out(r["logitsP"]) for r in res.results], axis=0)
